# revision 1
# baseline (speedup 1.0000x reference)
"""Causal self-attention on 8 Trainium2 NeuronCores (Bass/Tile).

Problem shape (hardcoded): x [2, 2048, 1024], W_attn [1024, 3072],
b_attn [3072], W_proj [1024, 1024], b_proj [1024], 16 heads, hd=64.

Sharding: tensor-parallel over (batch, head-group). Core k handles
batch k//4 and heads 4*(k%4) .. 4*(k%4)+3 (two head-pairs). Each core
computes its 4 heads' attention and a partial output projection
(y_local @ W_proj[rows]) of shape [2048, 1024]; the host sums the four
partials per batch and adds b_proj.
"""

import sys

for _p in ("/opt/trn_rl_repo", "/root/.axon_site/_ro/trn_rl_repo"):
    if _p not in sys.path:
        sys.path.insert(0, _p)

import numpy as np

import concourse.bass as bass  # noqa: F401  (engine types)
import concourse.mybir as mybir
import concourse.tile as tile
from concourse import bacc
from concourse.bass_utils import run_bass_kernel_spmd

F32 = mybir.dt.float32
F32R = mybir.dt.float32r

B = 2
T = 2048
C = 1024
H = 16
HD = 64
NCORES = 8
HEADS_PER_CORE = 4  # two pairs
PAIRS = 2
NKT = T // 128       # 16 k-tiles per head
NST = T // 512       # 4 q-strips per head
CKT = C // 128       # 8 contraction tiles for C

_CACHE = {}


def _build(phases=(1, 2, 3)):
    """Build the SPMD Bass program (identical for all cores)."""
    nc = bacc.Bacc(None, target_bir_lowering=False)

    xt_d = nc.dram_tensor("xt", [C, T], F32R, kind="ExternalInput")
    wq_d = nc.dram_tensor("wq", [128, PAIRS, CKT, 128], F32R, kind="ExternalInput")
    wk_d = nc.dram_tensor("wk", [128, PAIRS, CKT, 128], F32R, kind="ExternalInput")
    wv_d = nc.dram_tensor("wv", [128, PAIRS, CKT, 128], F32R, kind="ExternalInput")
    wp_d = nc.dram_tensor("wp", [128, 2, C], F32R, kind="ExternalInput")
    id_d = nc.dram_tensor("ident", [128, 128], F32R, kind="ExternalInput")
    ones_d = nc.dram_tensor("ones", [128, 1], F32R, kind="ExternalInput")
    mask_d = nc.dram_tensor("mask", [128, 128], F32R, kind="ExternalInput")
    out_d = nc.dram_tensor("out", [T, C], F32, kind="ExternalOutput")

    with tile.TileContext(nc) as tc, (
        tc.tile_pool(name="const", bufs=1)
    ) as const, (
        tc.tile_pool(name="weights", bufs=1)
    ) as wpool, (
        tc.tile_pool(name="acts", bufs=1)
    ) as apool, (
        tc.tile_pool(name="xstream", bufs=16)
    ) as xpool, (
        tc.tile_pool(name="ptp", bufs=3)
    ) as ppool, (
        tc.tile_pool(name="evict", bufs=3)
    ) as epool, (
        tc.tile_pool(name="dram_bounce", bufs=1, space="DRAM")
    ) as dpool:
        with (
            tc.tile_pool(name="st_ps", bufs=1, space="PSUM") as st_ps,
            tc.tile_pool(name="y_ps", bufs=1, space="PSUM") as y_ps,
            tc.tile_pool(name="qkv_ps", bufs=1, space="PSUM") as qkv_ps,
        ):
            ident = const.tile([128, 128], F32R)
            mask_tri = const.tile([128, 128], F32R)
            nc.sync.dma_start(ident[:], id_d[:])
            nc.sync.dma_start(mask_tri[:], mask_d[:])

            wq = wpool.tile([128, PAIRS, CKT, 128], F32R)
            wk = wpool.tile([128, PAIRS, CKT, 128], F32R)
            wv = wpool.tile([128, PAIRS, CKT, 128], F32R)
            wp = wpool.tile([128, 2, C], F32R)
            # pair-0 weight loads first so its matmuls start early; pair-1
            # loads are issued after the first x strip (wp just before proj)
            nc.sync.dma_start(wq[:, 0], wq_d[:, 0])
            nc.sync.dma_start(wk[:, 0], wk_d[:, 0])
            nc.sync.dma_start(wv[:, 0], wv_d[:, 0])

            # activations kept resident in SBUF
            qt = apool.tile([128, PAIRS, T], F32R)   # q^T, heads stacked in pairs
            kt = apool.tile([128, PAIRS, T], F32R)   # k^T
            # v^T is dead after the phase-1b transposes; ytn is written only in
            # phase 2b — share one buffer (Tile serializes the WAR hazard).
            vt = apool.tile([128, PAIRS, T], F32R)   # v^T (pre-transpose)
            ytn = vt
            v_nat = [
                apool.tile([128, NKT, HD + 1], F32R, name=f"vnat{i}", tag=f"vnat{i}")
                for i in range(HEADS_PER_CORE)
            ]
            ytu = apool.tile([128, PAIRS, T], F32R)  # unnormalized y^T
            sums_dram = dpool.tile([4, NST, 512], F32)

            # ---- Phase 1: QKV projections (transposed outputs) ----
            # loop q-strips of T; stream x^T chunks [128, 512]
            def emit_qkv(s):
                xch = [None] * CKT
                for kc in range(CKT):
                    xc = xpool.tile([128, 512], F32R, name=f"xc_{s}_{kc}", tag="xc")
                    nc.sync.dma_start(xc[:], xt_d[kc * 128:(kc + 1) * 128, s * 512:(s + 1) * 512])
                    xch[kc] = xc
                if s == 0:
                    nc.sync.dma_start(wq[:, 1], wq_d[:, 1])
                    nc.sync.dma_start(wk[:, 1], wk_d[:, 1])
                    nc.sync.dma_start(wv[:, 1], wv_d[:, 1])
                for p in range(PAIRS):
                    for w_t, dest in ((wq, qt), (wk, kt), (wv, vt)):
                        ps = qkv_ps.tile([128, 512], F32, name=f"qkvps_{s}_{p}", tag=f"qkv{p}")
                        for kc in range(CKT):
                            nc.tensor.matmul(
                                ps[:],
                                w_t[:, p, kc, :],
                                xch[kc][:],
                                start=(kc == 0),
                                stop=(kc == CKT - 1),
                            )
                        nc.vector.tensor_copy(dest[:, p, s * 512:(s + 1) * 512], ps[:])
                    # v natural layout for this strip via PE transpose:
                    # 4 t-tiles per psum slot, one wide eviction
                    for h in range(2):
                        hh = 2 * p + h
                        pt = qkv_ps.tile([128, 4 * HD], F32R, name=f"vtp_{s}_{p}_{h}", tag=f"qkv{p}")
                        for i, t in enumerate(range(4 * s, 4 * s + 4)):
                            nc.tensor.transpose(
                                pt[:, i * HD:(i + 1) * HD],
                                vt[h * HD:(h + 1) * HD, p, t * 128:(t + 1) * 128],
                                ident[h * HD:(h + 1) * HD, h * HD:(h + 1) * HD],
                            )
                        nc.vector.tensor_copy(
                            v_nat[hh][:, 4 * s:4 * s + 4, 0:HD],
                            pt[:].rearrange("p (t d) -> p t d", t=4),
                        )

            for hh in range(HEADS_PER_CORE):
                nc.sync.dma_start(
                    v_nat[hh][:, :, HD:HD + 1],
                    ones_d[:].to_broadcast((128, NKT, 1)),
                )

            # ---- Phase 2: attention ----
            recip_dram = dpool.tile([4, NST, 512], F32)

            def emit_attn(s):
                n_k = 4 * s + 4  # k-tiles for this strip (causal)
                for p in range(PAIRS):
                    ytile = [
                        y_ps.tile([HD + 1, 512], F32, name=f"y_{p}_{s}_{h}", tag=f"y{h}")
                        for h in range(2)
                    ]
                    ngrp = (n_k + 1) // 2
                    for g in range(ngrp):
                        st = [
                            st_ps.tile([128, 1024], F32, name=f"st_{p}_{s}_{g}_{h}", tag="st", bufs=2)
                            for h in range(2)
                        ]
                        njj = min(2, n_k - 2 * g)
                        # S^T matmuls, interleaved across heads for row-group overlap
                        for jj in range(njj):
                            j = 2 * g + jj
                            c0 = max(0, 128 * (j - 4 * s))
                            for h in range(2):
                                nc.tensor.matmul(
                                    st[h][:, jj * 512 + c0:(jj + 1) * 512],
                                    kt[h * HD:(h + 1) * HD, p, j * 128:(j + 1) * 128],
                                    qt[h * HD:(h + 1) * HD, p, s * 512 + c0:(s + 1) * 512],
                                    start=True,
                                    stop=True,
                                )
                        # exp (full group; unwritten cols are never read downstream)
                        for h in range(2):
                            hh = 2 * p + h
                            ptile = ppool.tile([128, 1024], F32R, name=f"pt_{p}_{s}_{g}_{h}", tag="pt", bufs=5)
                            nc.scalar.activation(
                                ptile[:], st[h][:], mybir.ActivationFunctionType.Exp
                            )
                            for jj in range(njj):
                                j = 2 * g + jj
                                c0 = max(0, 128 * (j - 4 * s))
                                if j >= 4 * s:  # diagonal block: triangular mask
                                    blk = ptile[:, jj * 512 + c0:jj * 512 + c0 + 128]
                                    nc.gpsimd.tensor_mul(blk, blk, mask_tri[:])
                                nc.tensor.matmul(
                                    ytile[h][:, c0:512],
                                    v_nat[hh][:, j, :],
                                    ptile[:, jj * 512 + c0:(jj + 1) * 512],
                                    start=(j == 0),
                                    stop=(j == n_k - 1),
                                )
                    # evict y (rows 0:64) and sums (row 64)
                    for h in range(2):
                        r = 2 * p + h
                        nc.vector.tensor_copy(
                            ytu[h * HD:(h + 1) * HD, p, s * 512:(s + 1) * 512],
                            ytile[h][0:HD, :],
                        )
                        srow = ppool.tile([HD + 1, 512], F32, name=f"srow_{s}_{r}", tag="srow")
                        nc.vector.tensor_copy(srow[HD:HD + 1, :], ytile[h][HD:HD + 1, :])
                        nc.sync.dma_start(sums_dram[r:r + 1, s, :], srow[HD:HD + 1, :])

                # ---- per-strip normalization ----
                sums_s = ppool.tile([4, 512], F32, name=f"sums_{s}", tag="sums")
                recip_s = ppool.tile([4, 512], F32, name=f"recip_{s}", tag="recip")
                rscr_s = ppool.tile([4, 512], F32, name=f"rscr_{s}", tag="rscr")
                nc.sync.dma_start(sums_s[:], sums_dram[:, s, :])
                nc.vector.reciprocal_approx_accurate(recip_s[:], sums_s[:], rscr_s[:])
                nc.sync.dma_start(recip_dram[:, s, :], recip_s[:])
                for p in range(PAIRS):
                    for h in range(2):
                        r = 2 * p + h
                        rb = ppool.tile([128, 512], F32, name=f"rb_{s}_{r}", tag="rb")
                        nc.sync.dma_start(
                            rb[h * HD:(h + 1) * HD, :],
                            recip_dram[r:r + 1, s, :].to_broadcast((HD, 512)),
                        )
                        nc.vector.tensor_mul(
                            ytn[h * HD:(h + 1) * HD, p, s * 512:(s + 1) * 512],
                            ytu[h * HD:(h + 1) * HD, p, s * 512:(s + 1) * 512],
                            rb[h * HD:(h + 1) * HD, :].bitcast(F32R),
                        )

            # ---- skewed software pipeline: attention trails QKV by 1 strip ----
            if 1 in phases:
                emit_qkv(0)
            for s in range(NST) if 2 in phases else []:
                if s + 1 < NST and 1 in phases:
                    emit_qkv(s + 1)
                emit_attn(s)

        # ---- Phase 3: output projection (partial) ----
        with tc.tile_pool(name="o_ps", bufs=2, space="PSUM") as o_ps:
            for f in range(2):
                nc.sync.dma_start(wp[:, f, :], wp_d[:, f, :])
            for t in range(NKT) if 3 in phases else []:
                op = o_ps.tile([128, 1024], F32, name=f"op_{t}", tag="op")
                for f in range(2):
                    for n in range(2):
                        nc.tensor.matmul(
                            op[:, n * 512:(n + 1) * 512],
                            ytn[:, f, t * 128:(t + 1) * 128],
                            wp[:, f, n * 512:(n + 1) * 512],
                            start=(f == 0),
                            stop=(f == 1),
                        )
                ot = epool.tile([128, 1024], F32, name=f"ot_{t}", tag="ot")
                nc.vector.tensor_copy(ot[:], op[:])
                nc.sync.dma_start(out_d[t * 128:(t + 1) * 128, :], ot[:])

    nc.compile()
    return nc


def _prep_inputs(x, W_attn, b_attn, W_proj):
    """Per-core input maps. Core k: batch k//4, head-group k%4."""
    assert np.allclose(b_attn, 0.0), "nonzero b_attn not supported by this kernel"
    scale = 1.0 / np.sqrt(np.float32(HD))

    ident = np.eye(128, dtype=np.float32)
    ones = np.ones((128, 1), dtype=np.float32)
    mask = (np.arange(128)[:, None] <= np.arange(128)[None, :]).astype(np.float32)

    def lhsT_tiles(w):
        # [C, 128] -> [128, CKT, 128] with [p, t, c] = w[t*128+p, c]
        return np.ascontiguousarray(w.reshape(CKT, 128, 128).transpose(1, 0, 2))

    in_maps = []
    for core in range(NCORES):
        b = core // 4
        g = core % 4
        heads = [4 * g + i for i in range(HEADS_PER_CORE)]
        xt = np.ascontiguousarray(x[b].T)  # [C, T]

        def w_slice(base, hs, sc=1.0):
            cols = np.concatenate(
                [np.arange(base + h * HD, base + (h + 1) * HD) for h in hs]
            )
            return np.ascontiguousarray(W_attn[:, cols]) * sc

        wq = np.stack(
            [lhsT_tiles(w_slice(0, heads[2 * p:2 * p + 2], scale)) for p in range(PAIRS)], axis=1
        )  # [128, PAIRS, CKT, 128]
        wk = np.stack(
            [lhsT_tiles(w_slice(C, heads[2 * p:2 * p + 2])) for p in range(PAIRS)], axis=1
        )
        wv = np.stack(
            [lhsT_tiles(w_slice(2 * C, heads[2 * p:2 * p + 2])) for p in range(PAIRS)], axis=1
        )
        # W_proj rows for this head group: [256, C] -> [128, 2, C]
        wp_rows = W_proj[heads[0] * HD:(heads[-1] + 1) * HD, :]
        wp = np.ascontiguousarray(wp_rows.reshape(2, 128, C).transpose(1, 0, 2))

        in_maps.append(
            {
                "xt": np.ascontiguousarray(xt, dtype=np.float32),
                "wq": np.ascontiguousarray(wq, dtype=np.float32),
                "wk": np.ascontiguousarray(wk, dtype=np.float32),
                "wv": np.ascontiguousarray(wv, dtype=np.float32),
                "wp": np.ascontiguousarray(wp, dtype=np.float32),
                "ident": ident,
                "ones": ones,
                "mask": mask,
            }
        )
    return in_maps


def kernel(x, W_attn, b_attn, W_proj, b_proj, _want_results=False, _spmd_kwargs=None):
    x = np.asarray(x, dtype=np.float32)
    W_attn = np.asarray(W_attn, dtype=np.float32)
    b_attn = np.asarray(b_attn, dtype=np.float32)
    W_proj = np.asarray(W_proj, dtype=np.float32)
    b_proj = np.asarray(b_proj, dtype=np.float32)

    if "nc" not in _CACHE:
        _CACHE["nc"] = _build()
    nc = _CACHE["nc"]

    in_maps = _prep_inputs(x, W_attn, b_attn, W_proj)
    kw = dict(_spmd_kwargs or {})
    res = run_bass_kernel_spmd(nc, in_maps, list(range(NCORES)), **kw)

    out = np.zeros((B, T, C), dtype=np.float32)
    for core in range(NCORES):
        out[core // 4] += res.results[core]["out"]
    out += b_proj[None, None, :]
    if _want_results:
        return out, res
    return out



# revision 13
# speedup vs baseline: 1.2299x; 1.2299x over previous
"""Causal self-attention on 8 Trainium2 NeuronCores (Bass/Tile).

Problem shape (hardcoded): x [2, 2048, 1024], W_attn [1024, 3072],
b_attn [3072], W_proj [1024, 1024], 16 heads, hd=64.

Sharding: tensor-parallel over (batch, head-group). Core k handles
batch k//4 and heads 4*(k%4) .. 4*(k%4)+3 (two head-pairs). Each core
computes its 4 heads' attention and a partial output projection
(y_local @ W_proj[rows]) of shape [2048, 1024]; the host sums the four
partials per batch and adds b_proj.

Layout strategy (cost model charges matmuls by moving-dim columns only):
- Q^T, K^T computed transposed ([d, T], f32r) for the S^T matmuls.
- V computed in natural layout [keys, d] directly (x-tile stationary,
  Wv moving 256-wide), cast to bf16.
- S^T = K^T.T @ Q^T per 128-key tile; exp on scalar engine -> P^T bf16.
- y = P^T.T @ V with bf16 operands: out [128q, 64d] per (head, q-tile),
  only 64 moving cols (half the cycles of the transposed orientation).
  Row sums via 1-column matmuls against a bf16 ones vector.
- Normalize y in natural layout (per-partition scalar on DVE), then PE
  bf16 transpose back to y^T for the projection.
- Projection with bf16 ytn/W_proj, interleaved into strips 2-3 where
  the scalar engine (exp) is the per-strip bottleneck.
"""

import sys

for _p in ("/opt/trn_rl_repo", "/root/.axon_site/_ro/trn_rl_repo"):
    if _p not in sys.path:
        sys.path.insert(0, _p)

import ml_dtypes
import numpy as np

import concourse.bass as bass  # noqa: F401  (engine types)
import concourse.mybir as mybir
import concourse.tile as tile
from concourse import bacc
from concourse.bass_utils import run_bass_kernel_spmd

F32 = mybir.dt.float32
F32R = mybir.dt.float32r
BF16 = mybir.dt.bfloat16

B = 2
T = 2048
C = 1024
H = 16
HD = 64
NCORES = 8
HEADS_PER_CORE = 4
PAIRS = 2
NKT = T // 128       # 16 k-tiles / t-tiles
NST = T // 512       # 4 q-strips
CKT = C // 128       # 8 contraction chunks of C

_CACHE = {}


def _build(debug=False):
    """Build the SPMD Bass program (identical for all cores)."""
    nc = bacc.Bacc(None, target_bir_lowering=False)
    dbg = {}
    if debug:
        dbg["qt"] = nc.dram_tensor("dbg_qt", [128, PAIRS, T], F32R, kind="ExternalOutput")
        dbg["kt"] = nc.dram_tensor("dbg_kt", [128, PAIRS, T], F32R, kind="ExternalOutput")
        dbg["v"] = nc.dram_tensor("dbg_v", [128, NKT, HEADS_PER_CORE, HD], BF16, kind="ExternalOutput")
        dbg["pt"] = nc.dram_tensor("dbg_pt", [128, 1024], BF16, kind="ExternalOutput")
        dbg["sums"] = nc.dram_tensor("dbg_sums", [128, 16], F32, kind="ExternalOutput")
        dbg["yacc"] = nc.dram_tensor("dbg_yacc", [128, 8, HD], F32, kind="ExternalOutput")
        dbg["ytn"] = nc.dram_tensor("dbg_ytn", [128, PAIRS, T], BF16, kind="ExternalOutput")

    xt_d = nc.dram_tensor("xt", [C, T], F32R, kind="ExternalInput")
    wq_d = nc.dram_tensor("wq", [128, PAIRS, CKT, 128], F32R, kind="ExternalInput")
    wk_d = nc.dram_tensor("wk", [128, PAIRS, CKT, 128], F32R, kind="ExternalInput")
    wv_d = nc.dram_tensor("wv", [128, CKT, 256], F32R, kind="ExternalInput")
    wp_d = nc.dram_tensor("wp", [128, 2, C], BF16, kind="ExternalInput")
    id_d = nc.dram_tensor("identb", [128, 128], BF16, kind="ExternalInput")
    mask_d = nc.dram_tensor("maskb", [128, 128], BF16, kind="ExternalInput")
    out_d = nc.dram_tensor("out", [T, C], F32, kind="ExternalOutput")

    with tile.TileContext(nc) as tc, (
        tc.tile_pool(name="const", bufs=1)
    ) as const, (
        tc.tile_pool(name="weights", bufs=1)
    ) as wpool, (
        tc.tile_pool(name="acts", bufs=1)
    ) as apool, (
        tc.tile_pool(name="xstream", bufs=16)
    ) as xpool, (
        tc.tile_pool(name="ptp", bufs=18)
    ) as ppool, (
        tc.tile_pool(name="small", bufs=1)
    ) as spool, (
        tc.tile_pool(name="evict", bufs=3)
    ) as epool:
        with (
            tc.tile_pool(name="st_ps", bufs=1, space="PSUM") as st_ps,
            tc.tile_pool(name="y_ps", bufs=1, space="PSUM") as y_ps,
            tc.tile_pool(name="qkv_ps", bufs=1, space="PSUM") as qkv_ps,
            tc.tile_pool(name="misc_ps", bufs=1, space="PSUM") as misc_ps,
        ):
            identb = const.tile([128, 128], BF16)
            maskb = const.tile([128, 128], BF16)
            onesb = const.tile([128, 1], BF16)
            nc.gpsimd.memset(onesb[:], 1.0)

            wq = wpool.tile([128, PAIRS, CKT, 128], F32R)
            wk = wpool.tile([128, PAIRS, CKT, 128], F32R)
            wv = wpool.tile([128, CKT, 256], F32R)
            wp = wpool.tile([128, 2, C], BF16)
            nc.sync.dma_start(wq[:, 0], wq_d[:, 0])
            nc.sync.dma_start(wk[:, 0], wk_d[:, 0])

            # activations resident in SBUF
            qt = apool.tile([128, PAIRS, T], F32R)     # q^T (2 heads on partitions)
            kt = apool.tile([128, PAIRS, T], F32R)     # k^T
            v_nat = apool.tile([128, NKT, HEADS_PER_CORE, HD], BF16)  # V natural
            ytn = apool.tile([128, PAIRS, T], BF16)    # normalized y^T

            # one PSUM bank [128, 512] f32 shared by softmax sums (cols 0:16)
            # and the y^T transpose bounce regions (f32 cols 128:192 / 192:256
            # bitcast to bf16 [128,128] each)
            misc = misc_ps.tile([128, 512], F32)

            # ---- Phase 1: QKV projections ----
            def emit_qkv(s):
                xch = [None] * CKT
                for kc in range(CKT):
                    xc = xpool.tile([128, 512], F32R, name=f"xc_{s}_{kc}", tag="xc")
                    nc.sync.dma_start(xc[:], xt_d[kc * 128:(kc + 1) * 128, s * 512:(s + 1) * 512])
                    xch[kc] = xc
                if s == 0:
                    nc.sync.dma_start(wq[:, 1], wq_d[:, 1])
                    nc.sync.dma_start(wk[:, 1], wk_d[:, 1])
                    nc.sync.dma_start(wv[:], wv_d[:])
                    nc.sync.dma_start(identb[:], id_d[:])
                    nc.sync.dma_start(maskb[:], mask_d[:])
                if s == 1:
                    nc.sync.dma_start(wp[:], wp_d[:])
                for p in range(PAIRS):
                    for w_t, dest in ((wq, qt), (wk, kt)):
                        ps = qkv_ps.tile([128, 512], F32, name=f"qkvps_{s}_{p}_{0 if w_t is wq else 1}", tag=f"qkv{p}")
                        for kc in range(CKT):
                            nc.tensor.matmul(
                                ps[:],
                                w_t[:, p, kc, :],
                                xch[kc][:],
                                start=(kc == 0),
                                stop=(kc == CKT - 1),
                            )
                        nc.vector.tensor_copy(dest[:, p, s * 512:(s + 1) * 512], ps[:])
                # V natural: x-tile stationary, wv moving (256 cols)
                for ii in range(4):
                    t = 4 * s + ii
                    vp = qkv_ps.tile([128, 256], F32, name=f"vps_{s}_{ii}", tag=f"qkv{ii % 2}")
                    for kc in range(CKT):
                        nc.tensor.matmul(
                            vp[:],
                            xch[kc][:, ii * 128:(ii + 1) * 128],
                            wv[:, kc, :],
                            start=(kc == 0),
                            stop=(kc == CKT - 1),
                        )
                    nc.vector.tensor_copy(
                        v_nat[:, t, :, :],
                        vp[:].rearrange("p (h d) -> p h d", h=HEADS_PER_CORE),
                    )

            # ---- Phase 2: attention ----
            # y accumulators: one bank, 8 accs [128, 64] per pair (acc = 4h+ii)
            yacc = y_ps.tile([128, 8, HD], F32)

            deferred = []  # callables emitted after the next S group

            def flush_deferred():
                while deferred:
                    deferred.pop(0)()

            def emit_S(s, p, g, st):
                n_k = 4 * s + 4
                njj = min(2, n_k - 2 * g)
                for jj in range(njj):
                    j = 2 * g + jj
                    c0 = max(0, 128 * (j - 4 * s))
                    for h in range(2):
                        nc.tensor.matmul(
                            st[h][:, jj * 512 + c0:(jj + 1) * 512],
                            kt[h * HD:(h + 1) * HD, p, j * 128:(j + 1) * 128],
                            qt[h * HD:(h + 1) * HD, p, s * 512 + c0:(s + 1) * 512],
                            start=True,
                            stop=True,
                        )

            def emit_exp(s, p, g, st, pts):
                for h in range(2):
                    ptile = ppool.tile([128, 1024], BF16, name=f"pt_{p}_{s}_{g}_{h}", tag="pt")
                    nc.scalar.activation(
                        ptile[:], st[h][:], mybir.ActivationFunctionType.Exp
                    )
                    njj = min(2, 4 * s + 4 - 2 * g)
                    for jj in range(njj):
                        j = 2 * g + jj
                        if j >= 4 * s:  # diagonal block: triangular mask
                            c0 = 128 * (j - 4 * s)
                            blk = ptile[:, jj * 512 + c0:jj * 512 + c0 + 128]
                            nc.gpsimd.tensor_mul(blk, blk, maskb[:])
                    pts[g * 2 + h] = ptile
                    if debug and (s, p, g, h) == (0, 0, 0, 0):
                        nc.sync.dma_start(dbg["pt"][:], ptile[:])

            def emit_y(s, p, pts):
                """Per-accumulator chains: PSUM accumulation context is
                per-bank and only one group may be open per bank, so each
                (head, q-tile) runs its j-chain back-to-back. The y chain
                (yacc bank) and sums chain (misc bank) interleave safely."""
                for h in range(2):
                    hh = 2 * p + h
                    for ii in range(4):
                        acc = 4 * h + ii
                        for j in range(4 * s + ii + 1):
                            ptile = pts[(j // 2) * 2 + h]
                            jj = j % 2
                            st_flag = (j == 0)
                            sp_flag = (j == 4 * s + ii)
                            lhsT = ptile[:, jj * 512 + ii * 128:jj * 512 + (ii + 1) * 128]
                            nc.tensor.matmul(
                                yacc[:, acc, :],
                                lhsT,
                                v_nat[:, j, hh, :],
                                start=st_flag,
                                stop=sp_flag,
                            )
                            nc.tensor.matmul(
                                misc[:, 8 * p + acc:8 * p + acc + 1],
                                lhsT,
                                onesb[:],
                                start=st_flag,
                                stop=sp_flag,
                            )

            def emit_normalize(s, p):
                """recip of sums + per-partition scalar normalize -> yn bf16."""
                rtile = spool.tile([128, 8], F32, name=f"rt_{s}_{p}", tag="rt", bufs=2)
                rscr = spool.tile([128, 8], F32, name=f"rs_{s}_{p}", tag="rs", bufs=2)
                nc.vector.reciprocal_approx_accurate(
                    rtile[:], misc[:, 8 * p:8 * p + 8], rscr[:]
                )
                yns = [None] * 8
                for ii in range(4):
                    for h in range(2):
                        acc = 4 * h + ii
                        yn = spool.tile([128, HD], BF16, name=f"yn_{s}_{p}_{acc}", tag="yn", bufs=6)
                        nc.vector.tensor_scalar_mul(
                            yn[:], yacc[:, acc, :], rtile[:, acc:acc + 1]
                        )
                        yns[acc] = yn
                return yns

            def emit_transposes(s, p, yns):
                """PE bf16 transposes via misc bounce regions + DVE evicts."""
                for ii in range(4):
                    t = 4 * s + ii
                    reg = misc[:, 128 + 64 * (ii % 2):192 + 64 * (ii % 2)].bitcast(BF16)
                    for h in range(2):
                        nc.tensor.transpose(
                            reg[h * HD:(h + 1) * HD, :],
                            yns[4 * h + ii][:],
                            identb[:],
                        )
                    nc.vector.tensor_copy(
                        ytn[:, p, t * 128:(t + 1) * 128], reg[:]
                    )

            def emit_proj(t):
                """partial projection for t-tile t via qkv psum tags."""
                ot = epool.tile([128, 1024], F32, name=f"ot_{t}", tag="ot")
                for n in range(2):
                    op = qkv_ps.tile([128, 512], F32, name=f"op_{t}_{n}", tag=f"qkv{n}")
                    for f in range(2):
                        nc.tensor.matmul(
                            op[:],
                            ytn[:, f, t * 128:(t + 1) * 128],
                            wp[:, f, n * 512:(n + 1) * 512],
                            start=(f == 0),
                            stop=(f == 1),
                        )
                    nc.vector.tensor_copy(ot[:, n * 512:(n + 1) * 512], op[:])
                nc.sync.dma_start(out_d[t * 128:(t + 1) * 128, :], ot[:])

            def emit_attn(s, proj_tiles=()):
                ngrp = (4 * s + 4) // 2
                proj_q = list(proj_tiles)
                for p in range(PAIRS):
                    pts = [None] * (2 * ngrp)
                    for g in range(ngrp):
                        st = [
                            st_ps.tile([128, 1024], F32, name=f"st_{p}_{s}_{g}_{h}", tag="st", bufs=2)
                            for h in range(2)
                        ]
                        emit_S(s, p, g, st)
                        if g == 1:
                            flush_deferred()
                        if proj_q and g >= 2:
                            emit_proj(proj_q.pop(0))
                        emit_exp(s, p, g, st, pts)
                    emit_y(s, p, pts)
                    if debug and s == 0 and p == 0:
                        yd = epool.tile([128, 8, HD], F32, name="dbg_yacc_b", tag="ot")
                        nc.vector.tensor_copy(yd[:], yacc[:])
                        nc.sync.dma_start(dbg["yacc"][:], yd[:])
                    yns = emit_normalize(s, p)
                    if debug and s == 0 and p == 1:
                        sd = epool.tile([128, 16], F32, name="dbg_sums_b", tag="ot")
                        nc.vector.tensor_copy(sd[:], misc[:, 0:16])
                        nc.sync.dma_start(dbg["sums"][:], sd[:])
                    deferred.append(
                        (lambda s=s, p=p, yns=yns: emit_transposes(s, p, yns))
                    )
                for t in proj_q:
                    emit_proj(t)

            # ---- schedule: qkv prefetched 2 strips ahead; proj in s2/s3 ----
            emit_qkv(0)
            emit_qkv(1)
            emit_attn(0)
            emit_qkv(2)
            emit_attn(1)
            emit_qkv(3)
            emit_attn(2, proj_tiles=range(0, 6))
            emit_attn(3, proj_tiles=range(6, 12))
            flush_deferred()
            for t in range(12, 16):
                emit_proj(t)

            if debug:
                nc.sync.dma_start(dbg["qt"][:], qt[:])
                nc.sync.dma_start(dbg["kt"][:], kt[:])
                nc.sync.dma_start(dbg["v"][:], v_nat[:])
                nc.sync.dma_start(dbg["ytn"][:], ytn[:])

    nc.compile()
    return nc


def _prep_inputs(x, W_attn, b_attn, W_proj):
    """Per-core input maps. Core k: batch k//4, head-group k%4."""
    assert np.allclose(b_attn, 0.0), "nonzero b_attn not supported by this kernel"
    scale = 1.0 / np.sqrt(np.float32(HD))

    identb = np.eye(128, dtype=ml_dtypes.bfloat16)
    maskb = (np.arange(128)[:, None] <= np.arange(128)[None, :]).astype(
        ml_dtypes.bfloat16
    )

    def lhsT_tiles(w):
        # [C, 128] -> [128, CKT, 128] with [p, t, c] = w[t*128+p, c]
        return np.ascontiguousarray(w.reshape(CKT, 128, 128).transpose(1, 0, 2))

    in_maps = []
    for core in range(NCORES):
        b = core // 4
        g = core % 4
        heads = [4 * g + i for i in range(HEADS_PER_CORE)]
        xt = np.ascontiguousarray(x[b].T)  # [C, T]

        def w_slice(base, hs, sc=1.0):
            cols = np.concatenate(
                [np.arange(base + h * HD, base + (h + 1) * HD) for h in hs]
            )
            return np.ascontiguousarray(W_attn[:, cols]) * sc

        wq = np.stack(
            [lhsT_tiles(w_slice(0, heads[2 * p:2 * p + 2], scale)) for p in range(PAIRS)], axis=1
        )  # [128, PAIRS, CKT, 128]
        wk = np.stack(
            [lhsT_tiles(w_slice(C, heads[2 * p:2 * p + 2])) for p in range(PAIRS)], axis=1
        )
        # V natural: [C, 256] -> [128, CKT, 256] with [p, c, col] = w[c*128+p, col]
        wv_cols = w_slice(2 * C, heads)  # [C, 256]
        wv = np.ascontiguousarray(wv_cols.reshape(CKT, 128, 256).transpose(1, 0, 2))
        # W_proj rows for this head group: [256, C] -> [128, 2, C], bf16
        wp_rows = W_proj[heads[0] * HD:(heads[-1] + 1) * HD, :]
        wp = np.ascontiguousarray(
            wp_rows.reshape(2, 128, C).transpose(1, 0, 2)
        ).astype(ml_dtypes.bfloat16)

        in_maps.append(
            {
                "xt": np.ascontiguousarray(xt, dtype=np.float32),
                "wq": np.ascontiguousarray(wq, dtype=np.float32),
                "wk": np.ascontiguousarray(wk, dtype=np.float32),
                "wv": np.ascontiguousarray(wv, dtype=np.float32),
                "wp": wp,
                "identb": identb,
                "maskb": maskb,
            }
        )
    return in_maps


def kernel(x, W_attn, b_attn, W_proj, b_proj, _want_results=False, _spmd_kwargs=None):
    x = np.asarray(x, dtype=np.float32)
    W_attn = np.asarray(W_attn, dtype=np.float32)
    b_attn = np.asarray(b_attn, dtype=np.float32)
    W_proj = np.asarray(W_proj, dtype=np.float32)
    b_proj = np.asarray(b_proj, dtype=np.float32)

    if "nc" not in _CACHE:
        _CACHE["nc"] = _build()
    nc = _CACHE["nc"]

    in_maps = _prep_inputs(x, W_attn, b_attn, W_proj)
    kw = dict(_spmd_kwargs or {})
    res = run_bass_kernel_spmd(nc, in_maps, list(range(NCORES)), **kw)

    out = np.zeros((B, T, C), dtype=np.float32)
    for core in range(NCORES):
        out[core // 4] += res.results[core]["out"]
    out += b_proj[None, None, :]
    if _want_results:
        return out, res
    return out


# revision 26
# speedup vs baseline: 1.2846x; 1.0445x over previous
"""Causal self-attention on 8 Trainium2 NeuronCores (Bass/Tile).

Problem shape (hardcoded): x [2, 2048, 1024], W_attn [1024, 3072],
b_attn [3072], W_proj [1024, 1024], 16 heads, hd=64.

Sharding: tensor-parallel over (batch, head-group). Core k handles
batch k//4 and heads 4*(k%4) .. 4*(k%4)+3 (two head-pairs). Each core
computes its 4 heads' attention and a partial output projection
(y_local @ W_proj[rows]) of shape [2048, 1024]; the host sums the four
partials per batch and adds b_proj.

Layout strategy (cost model charges matmuls by moving-dim columns only):
- Q^T, K^T computed transposed ([d, T], f32r) for the S^T matmuls.
- V computed in natural layout [keys, d] directly (x-tile stationary,
  Wv moving 256-wide), cast to bf16.
- S^T = K^T.T @ Q^T per 128-key tile; exp on scalar engine -> P^T bf16.
- y = P^T.T @ V with bf16 operands: out [128q, 64d] per (head, q-tile),
  only 64 moving cols (half the cycles of the transposed orientation).
  Row sums via 1-column matmuls against a bf16 ones vector.
- Normalize y in natural layout (per-partition scalar on DVE), then PE
  bf16 transpose back to y^T for the projection.
- Projection with bf16 ytn/W_proj, interleaved into strips 2-3 where
  the scalar engine (exp) is the per-strip bottleneck.
"""

import sys

for _p in ("/opt/trn_rl_repo", "/root/.axon_site/_ro/trn_rl_repo"):
    if _p not in sys.path:
        sys.path.insert(0, _p)

import ml_dtypes
import numpy as np

import concourse.bass as bass  # noqa: F401  (engine types)
import concourse.mybir as mybir
import concourse.tile as tile
from concourse import bacc
from concourse.bass_utils import run_bass_kernel_spmd

F32 = mybir.dt.float32
F32R = mybir.dt.float32r
BF16 = mybir.dt.bfloat16

B = 2
T = 2048
C = 1024
H = 16
HD = 64
NCORES = 8
HEADS_PER_CORE = 4
PAIRS = 2
NKT = T // 128       # 16 k-tiles / t-tiles
NST = T // 512       # 4 q-strips
CKT = C // 128       # 8 contraction chunks of C

_CACHE = {}


def _build(debug=False):
    """Build the SPMD Bass program (identical for all cores)."""
    nc = bacc.Bacc(None, target_bir_lowering=False)
    dbg = {}
    if debug:
        dbg["qt"] = nc.dram_tensor("dbg_qt", [128, PAIRS, T], F32R, kind="ExternalOutput")
        dbg["kt"] = nc.dram_tensor("dbg_kt", [128, PAIRS, T], F32R, kind="ExternalOutput")
        dbg["v"] = nc.dram_tensor("dbg_v", [128, NKT, HEADS_PER_CORE, HD], BF16, kind="ExternalOutput")
        dbg["pt"] = nc.dram_tensor("dbg_pt", [128, 1024], BF16, kind="ExternalOutput")
        dbg["yacc"] = nc.dram_tensor("dbg_yacc", [128, 2, 4, 66], F32, kind="ExternalOutput")
        dbg["ytn"] = nc.dram_tensor("dbg_ytn", [128, PAIRS, T], BF16, kind="ExternalOutput")

    xt_d = nc.dram_tensor("xt", [C, T], F32R, kind="ExternalInput")
    wq_d = nc.dram_tensor("wq", [128, PAIRS, CKT, 128], F32R, kind="ExternalInput")
    wk_d = nc.dram_tensor("wk", [128, PAIRS, CKT, 128], F32R, kind="ExternalInput")
    wv_d = nc.dram_tensor("wv", [128, CKT, 256], F32R, kind="ExternalInput")
    wp_d = nc.dram_tensor("wp", [128, 2, C], BF16, kind="ExternalInput")
    id_d = nc.dram_tensor("identb", [128, 128], BF16, kind="ExternalInput")
    mask_d = nc.dram_tensor("maskb", [128, 128], BF16, kind="ExternalInput")
    out_d = nc.dram_tensor("out", [T, C], F32, kind="ExternalOutput")

    with tile.TileContext(nc) as tc, (
        tc.tile_pool(name="const", bufs=1)
    ) as const, (
        tc.tile_pool(name="weights", bufs=1)
    ) as wpool, (
        tc.tile_pool(name="acts", bufs=1)
    ) as apool, (
        tc.tile_pool(name="xstream", bufs=16)
    ) as xpool, (
        tc.tile_pool(name="ptp", bufs=18)
    ) as ppool, (
        tc.tile_pool(name="small", bufs=1)
    ) as spool, (
        tc.tile_pool(name="evict", bufs=3)
    ) as epool:
        with (
            tc.tile_pool(name="st_ps", bufs=1, space="PSUM") as st_ps,
            tc.tile_pool(name="y_ps", bufs=1, space="PSUM") as y_ps,
            tc.tile_pool(name="qkv_ps", bufs=1, space="PSUM") as qkv_ps,
            tc.tile_pool(name="misc_ps", bufs=1, space="PSUM") as misc_ps,
        ):
            identb = const.tile([128, 128], BF16)
            maskb = const.tile([128, 128], BF16)

            wq = wpool.tile([128, PAIRS, CKT, 128], F32R)
            wk = wpool.tile([128, PAIRS, CKT, 128], F32R)
            wv = wpool.tile([128, CKT, 256], F32R)
            wp = wpool.tile([128, 2, C], BF16)
            nc.sync.dma_start(wq[:, 0], wq_d[:, 0])

            # activations resident in SBUF
            qt = apool.tile([128, PAIRS, T], F32R)     # q^T (2 heads on partitions)
            kt = apool.tile([128, PAIRS, T], F32R)     # k^T
            # V natural [keys, d] + ones column at d=64 (sums fold into the
            # y matmul); 66-wide so each acc's 66 f32 cols stay 8B-aligned
            v_nat = apool.tile([128, NKT, HEADS_PER_CORE, 66], BF16)
            ytn = apool.tile([128, PAIRS, T], BF16)    # normalized y^T
            nc.gpsimd.memset(v_nat[:, :, :, 64:66], 1.0)

            # Accumulator banks: PSUM accumulation context is per-bank and
            # only one group may be open per bank at a time. h0's 4 accs
            # ([128, 4, 66] f32) live in ybank0; h1's in misc cols 0:264.
            # misc also hosts the y^T transpose bounce regions (f32 cols
            # 288:352 / 352:416, bitcast to bf16 [128,128] each).
            misc = misc_ps.tile([128, 512], F32)
            ybank0 = y_ps.tile([128, 4, 66], F32)

            # ---- Phase 1: QKV projections ----
            def emit_qkv(s):
                xch = [None] * CKT
                for kc in range(CKT):
                    xc = xpool.tile([128, 512], F32R, name=f"xc_{s}_{kc}", tag="xc")
                    nc.sync.dma_start(xc[:], xt_d[kc * 128:(kc + 1) * 128, s * 512:(s + 1) * 512])
                    xch[kc] = xc
                if s == 0:
                    nc.sync.dma_start(wk[:, 0], wk_d[:, 0])
                    nc.sync.dma_start(wq[:, 1], wq_d[:, 1])
                    nc.sync.dma_start(wk[:, 1], wk_d[:, 1])
                    nc.sync.dma_start(wv[:], wv_d[:])
                    nc.sync.dma_start(identb[:], id_d[:])
                    nc.sync.dma_start(maskb[:], mask_d[:])
                if s == 1:
                    nc.sync.dma_start(wp[:], wp_d[:])
                for p in range(PAIRS):
                    for w_t, dest in ((wq, qt), (wk, kt)):
                        ps = qkv_ps.tile([128, 512], F32, name=f"qkvps_{s}_{p}_{0 if w_t is wq else 1}", tag=f"qkv{p}")
                        for kc in range(CKT):
                            nc.tensor.matmul(
                                ps[:],
                                w_t[:, p, kc, :],
                                xch[kc][:],
                                start=(kc == 0),
                                stop=(kc == CKT - 1),
                            )
                        nc.vector.tensor_copy(dest[:, p, s * 512:(s + 1) * 512], ps[:])
                # V natural: x-tile stationary, wv moving (256 cols)
                for ii in range(4):
                    t = 4 * s + ii
                    vp = qkv_ps.tile([128, 256], F32, name=f"vps_{s}_{ii}", tag=f"qkv{ii % 2}")
                    for kc in range(CKT):
                        nc.tensor.matmul(
                            vp[:],
                            xch[kc][:, ii * 128:(ii + 1) * 128],
                            wv[:, kc, :],
                            start=(kc == 0),
                            stop=(kc == CKT - 1),
                        )
                    nc.vector.tensor_copy(
                        v_nat[:, t, :, 0:64],
                        vp[:].rearrange("p (h d) -> p h d", h=HEADS_PER_CORE),
                    )

            # ---- Phase 2: attention ----
            deferred = []  # callables emitted after the next S group

            def flush_deferred():
                while deferred:
                    deferred.pop(0)()

            def emit_S(s, p, g, st):
                n_k = 4 * s + 4
                njj = min(2, n_k - 2 * g)
                for jj in range(njj):
                    j = 2 * g + jj
                    c0 = max(0, 128 * (j - 4 * s))
                    for h in range(2):
                        nc.tensor.matmul(
                            st[h][:, jj * 512 + c0:(jj + 1) * 512],
                            kt[h * HD:(h + 1) * HD, p, j * 128:(j + 1) * 128],
                            qt[h * HD:(h + 1) * HD, p, s * 512 + c0:(s + 1) * 512],
                            start=True,
                            stop=True,
                        )

            def emit_exp(s, p, g, st, pts):
                # cols below the first live column (diagonal c0 of j=2g) are
                # never read downstream; skip them in the activation
                trim = max(0, 128 * (2 * g - 4 * s))
                for h in range(2):
                    ptile = ppool.tile([128, 1024], BF16, name=f"pt_{p}_{s}_{g}_{h}", tag="pt")
                    nc.scalar.activation(
                        ptile[:, trim:], st[h][:, trim:], mybir.ActivationFunctionType.Exp
                    )
                    njj = min(2, 4 * s + 4 - 2 * g)
                    for jj in range(njj):
                        j = 2 * g + jj
                        if j >= 4 * s:  # diagonal block: triangular mask
                            c0 = 128 * (j - 4 * s)
                            blk = ptile[:, jj * 512 + c0:jj * 512 + c0 + 128]
                            nc.gpsimd.tensor_mul(blk, blk, maskb[:])
                    pts[g * 2 + h] = ptile
                    if debug and (s, p, g, h) == (0, 0, 0, 0):
                        nc.sync.dma_start(dbg["pt"][:], ptile[:])

            yview = [ybank0, misc[:, 0:264].rearrange("p (i c) -> p i c", i=4)]

            def emit_y(s, p, pts):
                """Per-accumulator chains: PSUM accumulation context is
                per-bank and only one group may be open per bank, so each
                (head, q-tile) runs its j-chain back-to-back; h0 (ybank0)
                and h1 (misc) chains interleave across the two banks."""
                for ii in range(4):
                    for j in range(4 * s + ii + 1):
                        jj = j % 2
                        st_flag = (j == 0)
                        sp_flag = (j == 4 * s + ii)
                        for h in range(2):
                            ptile = pts[(j // 2) * 2 + h]
                            nc.tensor.matmul(
                                yview[h][:, ii, :],
                                ptile[:, jj * 512 + ii * 128:jj * 512 + (ii + 1) * 128],
                                v_nat[:, j, 2 * p + h, :],
                                start=st_flag,
                                stop=sp_flag,
                            )

            def emit_normalize(s, p):
                """recip of sums + per-partition scalar normalize -> yn bf16."""
                rtile = spool.tile([128, 2, 4], F32, name=f"rt_{s}_{p}", tag="rt", bufs=2)
                rscr = spool.tile([128, 2, 4], F32, name=f"rs_{s}_{p}", tag="rs", bufs=2)
                for h in range(2):
                    nc.vector.reciprocal_approx_accurate(
                        rtile[:, h, :], yview[h][:, :, 64:65], rscr[:, h, :]
                    )
                yns = [None] * 8
                for ii in range(4):
                    for h in range(2):
                        acc = 4 * h + ii
                        yn = spool.tile([128, HD], BF16, name=f"yn_{s}_{p}_{acc}", tag="yn", bufs=10)
                        nc.vector.tensor_scalar_mul(
                            yn[:], yview[h][:, ii, 0:64], rtile[:, h, ii:ii + 1]
                        )
                        yns[acc] = yn
                return yns

            def emit_transposes(s, p, yns):
                """PE bf16 transposes via misc bounce regions + DVE evicts."""
                for ii in range(4):
                    t = 4 * s + ii
                    reg = misc[:, 288 + 64 * (ii % 2):352 + 64 * (ii % 2)].bitcast(BF16)
                    for h in range(2):
                        nc.tensor.transpose(
                            reg[h * HD:(h + 1) * HD, :],
                            yns[4 * h + ii][:],
                            identb[:],
                        )
                    nc.vector.tensor_copy(
                        ytn[:, p, t * 128:(t + 1) * 128], reg[:]
                    )

            def emit_proj(t):
                """partial projection for t-tile t via qkv psum tags."""
                ot = epool.tile([128, 1024], F32, name=f"ot_{t}", tag="ot")
                for n in range(2):
                    op = qkv_ps.tile([128, 512], F32, name=f"op_{t}_{n}", tag=f"qkv{n}")
                    for f in range(2):
                        nc.tensor.matmul(
                            op[:],
                            ytn[:, f, t * 128:(t + 1) * 128],
                            wp[:, f, n * 512:(n + 1) * 512],
                            start=(f == 0),
                            stop=(f == 1),
                        )
                    nc.vector.tensor_copy(ot[:, n * 512:(n + 1) * 512], op[:])
                nc.sync.dma_start(out_d[t * 128:(t + 1) * 128, :], ot[:])

            def emit_attn(s, proj_tiles=()):
                ngrp = (4 * s + 4) // 2
                proj_q = list(proj_tiles)
                for p in range(PAIRS):
                    pts = [None] * (2 * ngrp)
                    for g in range(ngrp):
                        st = [
                            st_ps.tile([128, 1024], F32, name=f"st_{p}_{s}_{g}_{h}", tag="st", bufs=2)
                            for h in range(2)
                        ]
                        emit_S(s, p, g, st)
                        if g == 1:
                            flush_deferred()
                        if proj_q and g >= 2:
                            emit_proj(proj_q.pop(0))
                        emit_exp(s, p, g, st, pts)
                    emit_y(s, p, pts)
                    if debug and s == 0 and p == 0:
                        yd = epool.tile([128, 2, 4, 66], F32, name="dbg_yacc_b", tag="ot")
                        for h in range(2):
                            nc.vector.tensor_copy(yd[:, h], yview[h][:])
                        nc.sync.dma_start(dbg["yacc"][:], yd[:])
                    yns = emit_normalize(s, p)
                    deferred.append(
                        (lambda s=s, p=p, yns=yns: emit_transposes(s, p, yns))
                    )
                for t in proj_q:
                    emit_proj(t)

            # ---- schedule: qkv prefetched 2 strips ahead; proj in s2/s3 ----
            emit_qkv(0)
            emit_qkv(1)
            emit_attn(0)
            emit_qkv(2)
            emit_attn(1)
            emit_qkv(3)
            emit_attn(2, proj_tiles=range(0, 6))
            emit_attn(3, proj_tiles=range(6, 12))
            flush_deferred()
            for t in range(12, 16):
                emit_proj(t)

            if debug:
                nc.sync.dma_start(dbg["qt"][:], qt[:])
                nc.sync.dma_start(dbg["kt"][:], kt[:])
                nc.sync.dma_start(dbg["v"][:], v_nat[:, :, :, 0:64])
                nc.sync.dma_start(dbg["ytn"][:], ytn[:])

    nc.compile()
    return nc


def _prep_inputs(x, W_attn, b_attn, W_proj):
    """Per-core input maps. Core k: batch k//4, head-group k%4."""
    assert np.allclose(b_attn, 0.0), "nonzero b_attn not supported by this kernel"
    scale = 1.0 / np.sqrt(np.float32(HD))

    identb = np.eye(128, dtype=ml_dtypes.bfloat16)
    maskb = (np.arange(128)[:, None] <= np.arange(128)[None, :]).astype(
        ml_dtypes.bfloat16
    )

    def lhsT_tiles(w):
        # [C, 128] -> [128, CKT, 128] with [p, t, c] = w[t*128+p, c]
        return np.ascontiguousarray(w.reshape(CKT, 128, 128).transpose(1, 0, 2))

    in_maps = []
    for core in range(NCORES):
        b = core // 4
        g = core % 4
        heads = [4 * g + i for i in range(HEADS_PER_CORE)]
        xt = np.ascontiguousarray(x[b].T)  # [C, T]

        def w_slice(base, hs, sc=1.0):
            cols = np.concatenate(
                [np.arange(base + h * HD, base + (h + 1) * HD) for h in hs]
            )
            return np.ascontiguousarray(W_attn[:, cols]) * sc

        wq = np.stack(
            [lhsT_tiles(w_slice(0, heads[2 * p:2 * p + 2], scale)) for p in range(PAIRS)], axis=1
        )  # [128, PAIRS, CKT, 128]
        wk = np.stack(
            [lhsT_tiles(w_slice(C, heads[2 * p:2 * p + 2])) for p in range(PAIRS)], axis=1
        )
        # V natural: [C, 256] -> [128, CKT, 256] with [p, c, col] = w[c*128+p, col]
        wv_cols = w_slice(2 * C, heads)  # [C, 256]
        wv = np.ascontiguousarray(wv_cols.reshape(CKT, 128, 256).transpose(1, 0, 2))
        # W_proj rows for this head group: [256, C] -> [128, 2, C], bf16
        wp_rows = W_proj[heads[0] * HD:(heads[-1] + 1) * HD, :]
        wp = np.ascontiguousarray(
            wp_rows.reshape(2, 128, C).transpose(1, 0, 2)
        ).astype(ml_dtypes.bfloat16)

        in_maps.append(
            {
                "xt": np.ascontiguousarray(xt, dtype=np.float32),
                "wq": np.ascontiguousarray(wq, dtype=np.float32),
                "wk": np.ascontiguousarray(wk, dtype=np.float32),
                "wv": np.ascontiguousarray(wv, dtype=np.float32),
                "wp": wp,
                "identb": identb,
                "maskb": maskb,
            }
        )
    return in_maps


def kernel(x, W_attn, b_attn, W_proj, b_proj, _want_results=False, _spmd_kwargs=None):
    x = np.asarray(x, dtype=np.float32)
    W_attn = np.asarray(W_attn, dtype=np.float32)
    b_attn = np.asarray(b_attn, dtype=np.float32)
    W_proj = np.asarray(W_proj, dtype=np.float32)
    b_proj = np.asarray(b_proj, dtype=np.float32)

    if "nc" not in _CACHE:
        _CACHE["nc"] = _build()
    nc = _CACHE["nc"]

    in_maps = _prep_inputs(x, W_attn, b_attn, W_proj)
    kw = dict(_spmd_kwargs or {})
    res = run_bass_kernel_spmd(nc, in_maps, list(range(NCORES)), **kw)

    out = np.zeros((B, T, C), dtype=np.float32)
    for core in range(NCORES):
        out[core // 4] += res.results[core]["out"]
    out += b_proj[None, None, :]
    if _want_results:
        return out, res
    return out


# revision 32
# speedup vs baseline: 1.3471x; 1.0487x over previous
"""Causal self-attention on 8 Trainium2 NeuronCores (Bass/Tile).

Problem shape (hardcoded): x [2, 2048, 1024], W_attn [1024, 3072],
b_attn [3072], W_proj [1024, 1024], 16 heads, hd=64.

Sharding: tensor-parallel over (batch, head-group). Core k handles
batch k//4 and heads 4*(k%4) .. 4*(k%4)+3 (two head-pairs). Each core
computes its 4 heads' attention and a partial output projection
(y_local @ W_proj[rows]) of shape [2048, 1024]; the host sums the four
partials per batch and adds b_proj.

Layout strategy (cost model charges matmuls by moving-dim columns only):
- Q^T, K^T computed transposed ([d, T], f32r) for the S^T matmuls.
- V computed in natural layout [keys, d] directly (x-tile stationary,
  Wv moving 256-wide), cast to bf16.
- S^T = K^T.T @ Q^T per 128-key tile; exp on scalar engine -> P^T bf16.
- y = P^T.T @ V with bf16 operands: out [128q, 64d] per (head, q-tile),
  only 64 moving cols (half the cycles of the transposed orientation).
  Row sums via 1-column matmuls against a bf16 ones vector.
- Normalize y in natural layout (per-partition scalar on DVE), then PE
  bf16 transpose back to y^T for the projection.
- Projection with bf16 ytn/W_proj, interleaved into strips 2-3 where
  the scalar engine (exp) is the per-strip bottleneck.
"""

import sys

for _p in ("/opt/trn_rl_repo", "/root/.axon_site/_ro/trn_rl_repo"):
    if _p not in sys.path:
        sys.path.insert(0, _p)

import ml_dtypes
import numpy as np

import concourse.bass as bass  # noqa: F401  (engine types)
import concourse.mybir as mybir
import concourse.tile as tile
from concourse import bacc
from concourse.bass_utils import run_bass_kernel_spmd

F32 = mybir.dt.float32
F32R = mybir.dt.float32r
BF16 = mybir.dt.bfloat16

B = 2
T = 2048
C = 1024
H = 16
HD = 64
NCORES = 8
HEADS_PER_CORE = 4
PAIRS = 2
NKT = T // 128       # 16 k-tiles / t-tiles
NST = T // 512       # 4 q-strips
CKT = C // 128       # 8 contraction chunks of C

_CACHE = {}
DISABLE_PROJ_FILLER = True


def _build(debug=False):
    """Build the SPMD Bass program (identical for all cores)."""
    nc = bacc.Bacc(None, target_bir_lowering=False)
    dbg = {}
    if debug:
        dbg["qt"] = nc.dram_tensor("dbg_qt", [128, PAIRS, T], F32R, kind="ExternalOutput")
        dbg["kt"] = nc.dram_tensor("dbg_kt", [128, PAIRS, T], F32R, kind="ExternalOutput")
        dbg["v"] = nc.dram_tensor("dbg_v", [128, NKT, HEADS_PER_CORE, HD], BF16, kind="ExternalOutput")
        dbg["pt"] = nc.dram_tensor("dbg_pt", [128, 1024], BF16, kind="ExternalOutput")
        dbg["yacc"] = nc.dram_tensor("dbg_yacc", [128, 2, 4, 66], F32, kind="ExternalOutput")
        dbg["ytn"] = nc.dram_tensor("dbg_ytn", [128, PAIRS, T], BF16, kind="ExternalOutput")

    xt_d = nc.dram_tensor("xt", [C, T], F32R, kind="ExternalInput")
    wq_d = nc.dram_tensor("wq", [128, PAIRS, CKT, 128], F32R, kind="ExternalInput")
    wk_d = nc.dram_tensor("wk", [128, PAIRS, CKT, 128], F32R, kind="ExternalInput")
    wv_d = nc.dram_tensor("wv", [128, CKT, 256], F32R, kind="ExternalInput")
    wp_d = nc.dram_tensor("wp", [128, 2, C], BF16, kind="ExternalInput")
    id_d = nc.dram_tensor("identb", [128, 128], BF16, kind="ExternalInput")
    mask_d = nc.dram_tensor("maskb", [128, 128], BF16, kind="ExternalInput")
    out_d = nc.dram_tensor("out", [T, C], F32, kind="ExternalOutput")

    with tile.TileContext(nc) as tc, (
        tc.tile_pool(name="const", bufs=1)
    ) as const, (
        tc.tile_pool(name="weights", bufs=1)
    ) as wpool, (
        tc.tile_pool(name="acts", bufs=1)
    ) as apool, (
        tc.tile_pool(name="xstream", bufs=16)
    ) as xpool, (
        tc.tile_pool(name="ptp", bufs=18)
    ) as ppool, (
        tc.tile_pool(name="small", bufs=1)
    ) as spool, (
        tc.tile_pool(name="evict", bufs=3)
    ) as epool:
        with (
            tc.tile_pool(name="st_ps", bufs=1, space="PSUM") as st_ps,
            tc.tile_pool(name="y_ps", bufs=1, space="PSUM") as y_ps,
            tc.tile_pool(name="qkv_ps", bufs=1, space="PSUM") as qkv_ps,
            tc.tile_pool(name="misc_ps", bufs=1, space="PSUM") as misc_ps,
        ):
            identb = const.tile([128, 128], BF16)
            maskb = const.tile([128, 128], BF16)

            wq = wpool.tile([128, PAIRS, CKT, 128], F32R)
            wk = wpool.tile([128, PAIRS, CKT, 128], F32R)
            wv = wpool.tile([128, CKT, 256], F32R)
            wp = wpool.tile([128, 2, C], BF16)
            nc.sync.dma_start(wq[:, 0], wq_d[:, 0])

            # activations resident in SBUF
            qt = apool.tile([128, PAIRS, T], F32R)     # q^T (2 heads on partitions)
            kt = apool.tile([128, PAIRS, T], F32R)     # k^T
            # V natural [keys, d] + ones column at d=64 (sums fold into the
            # y matmul); 66-wide so each acc's 66 f32 cols stay 8B-aligned
            v_nat = apool.tile([128, NKT, HEADS_PER_CORE, 66], BF16)
            ytn = apool.tile([128, PAIRS, T], BF16)    # normalized y^T
            nc.gpsimd.memset(v_nat[:, :, :, 64:66], 1.0)

            # Accumulator banks: PSUM accumulation context is per-bank and
            # only one group may be open per bank at a time. h0's 4 accs
            # ([128, 4, 66] f32) live in ybank0; h1's in misc cols 0:264.
            # misc also hosts the y^T transpose bounce regions (f32 cols
            # 288:352 / 352:416, bitcast to bf16 [128,128] each).
            misc = misc_ps.tile([128, 512], F32)
            ybank0 = y_ps.tile([128, 4, 66], F32)

            # ---- Phase 1: QKV projections, split into psum-group "units"
            # that the attention loop pops as PE filler between S groups ----
            def qkv_units(s):
                """Issue x DMAs for strip s now; return PE work units."""
                xch = [None] * CKT
                for kc in range(CKT):
                    xc = xpool.tile([128, 512], F32R, name=f"xc_{s}_{kc}", tag="xc")
                    nc.sync.dma_start(xc[:], xt_d[kc * 128:(kc + 1) * 128, s * 512:(s + 1) * 512])
                    xch[kc] = xc
                if s == 0:
                    nc.sync.dma_start(wk[:, 0], wk_d[:, 0])
                    nc.sync.dma_start(wq[:, 1], wq_d[:, 1])
                    nc.sync.dma_start(wk[:, 1], wk_d[:, 1])
                    nc.sync.dma_start(wv[:], wv_d[:])
                    nc.sync.dma_start(identb[:], id_d[:])
                    nc.sync.dma_start(maskb[:], mask_d[:])
                if s == 1:
                    nc.sync.dma_start(wp[:], wp_d[:])

                def qk_unit(p, w_t, dest, kind):
                    ps = qkv_ps.tile([128, 512], F32, name=f"qkvps_{s}_{p}_{kind}", tag=f"qkv{p}")
                    for kc in range(CKT):
                        nc.tensor.matmul(
                            ps[:],
                            w_t[:, p, kc, :],
                            xch[kc][:],
                            start=(kc == 0),
                            stop=(kc == CKT - 1),
                        )
                    nc.vector.tensor_copy(dest[:, p, s * 512:(s + 1) * 512], ps[:])

                def v_unit(ii):
                    # V natural: x-tile stationary, wv moving (256 cols)
                    t = 4 * s + ii
                    vp = qkv_ps.tile([128, 256], F32, name=f"vps_{s}_{ii}", tag=f"qkv{ii % 2}")
                    for kc in range(CKT):
                        nc.tensor.matmul(
                            vp[:],
                            xch[kc][:, ii * 128:(ii + 1) * 128],
                            wv[:, kc, :],
                            start=(kc == 0),
                            stop=(kc == CKT - 1),
                        )
                    nc.vector.tensor_copy(
                        v_nat[:, t, :, 0:64],
                        vp[:].rearrange("p (h d) -> p h d", h=HEADS_PER_CORE),
                    )

                units = []
                for p in range(PAIRS):
                    units.append(lambda p=p: qk_unit(p, wq, qt, "q"))
                    units.append(lambda p=p: qk_unit(p, wk, kt, "k"))
                for ii in range(4):
                    units.append(lambda ii=ii: v_unit(ii))
                return units

            # ---- Phase 2: attention ----
            deferred = []  # callables emitted after the next S group

            def flush_deferred():
                while deferred:
                    deferred.pop(0)()

            def emit_S(s, p, g, st):
                n_k = 4 * s + 4
                njj = min(2, n_k - 2 * g)
                for jj in range(njj):
                    j = 2 * g + jj
                    c0 = max(0, 128 * (j - 4 * s))
                    for h in range(2):
                        nc.tensor.matmul(
                            st[h][:, jj * 512 + c0:(jj + 1) * 512],
                            kt[h * HD:(h + 1) * HD, p, j * 128:(j + 1) * 128],
                            qt[h * HD:(h + 1) * HD, p, s * 512 + c0:(s + 1) * 512],
                            start=True,
                            stop=True,
                        )

            def emit_exp(s, p, g, st, pts):
                # cols below the first live column (diagonal c0 of j=2g) are
                # never read downstream; skip them in the activation
                trim = max(0, 128 * (2 * g - 4 * s))
                for h in range(2):
                    ptile = ppool.tile([128, 1024], BF16, name=f"pt_{p}_{s}_{g}_{h}", tag="pt")
                    nc.scalar.activation(
                        ptile[:, trim:], st[h][:, trim:], mybir.ActivationFunctionType.Exp
                    )
                    njj = min(2, 4 * s + 4 - 2 * g)
                    for jj in range(njj):
                        j = 2 * g + jj
                        if j >= 4 * s:  # diagonal block: triangular mask
                            c0 = 128 * (j - 4 * s)
                            blk = ptile[:, jj * 512 + c0:jj * 512 + c0 + 128]
                            nc.gpsimd.tensor_mul(blk, blk, maskb[:])
                    pts[g * 2 + h] = ptile
                    if debug and (s, p, g, h) == (0, 0, 0, 0):
                        nc.sync.dma_start(dbg["pt"][:], ptile[:])

            yview = [ybank0, misc[:, 0:264].rearrange("p (i c) -> p i c", i=4)]

            def emit_y(s, p, pts):
                """Per-accumulator chains: PSUM accumulation context is
                per-bank and only one group may be open per bank, so each
                (head, q-tile) runs its j-chain back-to-back; h0 (ybank0)
                and h1 (misc) chains interleave across the two banks."""
                for ii in range(4):
                    for j in range(4 * s + ii + 1):
                        jj = j % 2
                        st_flag = (j == 0)
                        sp_flag = (j == 4 * s + ii)
                        for h in range(2):
                            ptile = pts[(j // 2) * 2 + h]
                            nc.tensor.matmul(
                                yview[h][:, ii, :],
                                ptile[:, jj * 512 + ii * 128:jj * 512 + (ii + 1) * 128],
                                v_nat[:, j, 2 * p + h, :],
                                start=st_flag,
                                stop=sp_flag,
                            )

            def emit_normalize(s, p):
                """recip of sums + per-partition scalar normalize -> yn bf16."""
                rtile = spool.tile([128, 2, 4], F32, name=f"rt_{s}_{p}", tag="rt", bufs=2)
                rscr = spool.tile([128, 2, 4], F32, name=f"rs_{s}_{p}", tag="rs", bufs=2)
                for h in range(2):
                    nc.vector.reciprocal_approx_accurate(
                        rtile[:, h, :], yview[h][:, :, 64:65], rscr[:, h, :]
                    )
                yns = [None] * 8
                for ii in range(4):
                    for h in range(2):
                        acc = 4 * h + ii
                        yn = spool.tile([128, HD], BF16, name=f"yn_{s}_{p}_{acc}", tag="yn", bufs=10)
                        nc.vector.tensor_scalar_mul(
                            yn[:], yview[h][:, ii, 0:64], rtile[:, h, ii:ii + 1]
                        )
                        yns[acc] = yn
                return yns

            def emit_transposes(s, p, yns):
                """PE bf16 transposes via misc bounce regions + DVE evicts."""
                for ii in range(4):
                    t = 4 * s + ii
                    reg = misc[:, 288 + 64 * (ii % 2):352 + 64 * (ii % 2)].bitcast(BF16)
                    for h in range(2):
                        nc.tensor.transpose(
                            reg[h * HD:(h + 1) * HD, :],
                            yns[4 * h + ii][:],
                            identb[:],
                        )
                    nc.vector.tensor_copy(
                        ytn[:, p, t * 128:(t + 1) * 128], reg[:]
                    )

            def emit_proj(t):
                """partial projection for t-tile t via qkv psum tags."""
                ot = epool.tile([128, 1024], F32, name=f"ot_{t}", tag="ot")
                for n in range(2):
                    op = qkv_ps.tile([128, 512], F32, name=f"op_{t}_{n}", tag=f"qkv{n}")
                    for f in range(2):
                        nc.tensor.matmul(
                            op[:],
                            ytn[:, f, t * 128:(t + 1) * 128],
                            wp[:, f, n * 512:(n + 1) * 512],
                            start=(f == 0),
                            stop=(f == 1),
                        )
                    nc.vector.tensor_copy(ot[:, n * 512:(n + 1) * 512], op[:])
                nc.sync.dma_start(out_d[t * 128:(t + 1) * 128, :], ot[:])

            filler = []  # ready PE work units popped between S groups

            def pop_filler(n):
                for _ in range(min(n, len(filler))):
                    filler.pop(0)()

            def emit_attn(s):
                ngrp = (4 * s + 4) // 2
                per_group = 2 if s <= 1 else 1
                for p in range(PAIRS):
                    pts = [None] * (2 * ngrp)
                    for g in range(ngrp):
                        st = [
                            st_ps.tile([128, 1024], F32, name=f"st_{p}_{s}_{g}_{h}", tag="st", bufs=2)
                            for h in range(2)
                        ]
                        emit_S(s, p, g, st)
                        if g == 1:
                            flush_deferred()
                            if p == 0 and s > 0 and not DISABLE_PROJ_FILLER:
                                # previous strip's ytn is complete
                                filler.extend(
                                    lambda t=t: emit_proj(t)
                                    for t in range(4 * (s - 1), 4 * s)
                                )
                        pop_filler(per_group)
                        emit_exp(s, p, g, st, pts)
                    emit_y(s, p, pts)
                    if debug and s == 0 and p == 0:
                        yd = epool.tile([128, 2, 4, 66], F32, name="dbg_yacc_b", tag="ot")
                        for h in range(2):
                            nc.vector.tensor_copy(yd[:, h], yview[h][:])
                        nc.sync.dma_start(dbg["yacc"][:], yd[:])
                    yns = emit_normalize(s, p)
                    deferred.append(
                        (lambda s=s, p=p, yns=yns: emit_transposes(s, p, yns))
                    )

            # ---- schedule: q/k of strip 0 up front; everything else (v
            # units, later strips' qkv, projection tiles) flows through the
            # filler queue so PE stays fed while the scalar engine runs exp ----
            units0 = qkv_units(0)
            for u in units0[:4]:
                u()                      # q/k of strip 0
            filler.extend(units0[4:])    # v units of strip 0
            for s in range(NST):
                if s + 1 < NST:
                    filler.extend(qkv_units(s + 1))
                emit_attn(s)
                # qkv units for strip s+1 must complete before attn(s+1)
                pop_filler(len(filler))
            flush_deferred()
            for t in (range(16) if DISABLE_PROJ_FILLER else range(12, 16)):
                emit_proj(t)

            if debug:
                nc.sync.dma_start(dbg["qt"][:], qt[:])
                nc.sync.dma_start(dbg["kt"][:], kt[:])
                nc.sync.dma_start(dbg["v"][:], v_nat[:, :, :, 0:64])
                nc.sync.dma_start(dbg["ytn"][:], ytn[:])

    nc.compile()
    return nc


def _prep_inputs(x, W_attn, b_attn, W_proj):
    """Per-core input maps. Core k: batch k//4, head-group k%4."""
    assert np.allclose(b_attn, 0.0), "nonzero b_attn not supported by this kernel"
    scale = 1.0 / np.sqrt(np.float32(HD))

    identb = np.eye(128, dtype=ml_dtypes.bfloat16)
    maskb = (np.arange(128)[:, None] <= np.arange(128)[None, :]).astype(
        ml_dtypes.bfloat16
    )

    def lhsT_tiles(w):
        # [C, 128] -> [128, CKT, 128] with [p, t, c] = w[t*128+p, c]
        return np.ascontiguousarray(w.reshape(CKT, 128, 128).transpose(1, 0, 2))

    in_maps = []
    for core in range(NCORES):
        b = core // 4
        g = core % 4
        heads = [4 * g + i for i in range(HEADS_PER_CORE)]
        xt = np.ascontiguousarray(x[b].T)  # [C, T]

        def w_slice(base, hs, sc=1.0):
            cols = np.concatenate(
                [np.arange(base + h * HD, base + (h + 1) * HD) for h in hs]
            )
            return np.ascontiguousarray(W_attn[:, cols]) * sc

        wq = np.stack(
            [lhsT_tiles(w_slice(0, heads[2 * p:2 * p + 2], scale)) for p in range(PAIRS)], axis=1
        )  # [128, PAIRS, CKT, 128]
        wk = np.stack(
            [lhsT_tiles(w_slice(C, heads[2 * p:2 * p + 2])) for p in range(PAIRS)], axis=1
        )
        # V natural: [C, 256] -> [128, CKT, 256] with [p, c, col] = w[c*128+p, col]
        wv_cols = w_slice(2 * C, heads)  # [C, 256]
        wv = np.ascontiguousarray(wv_cols.reshape(CKT, 128, 256).transpose(1, 0, 2))
        # W_proj rows for this head group: [256, C] -> [128, 2, C], bf16
        wp_rows = W_proj[heads[0] * HD:(heads[-1] + 1) * HD, :]
        wp = np.ascontiguousarray(
            wp_rows.reshape(2, 128, C).transpose(1, 0, 2)
        ).astype(ml_dtypes.bfloat16)

        in_maps.append(
            {
                "xt": np.ascontiguousarray(xt, dtype=np.float32),
                "wq": np.ascontiguousarray(wq, dtype=np.float32),
                "wk": np.ascontiguousarray(wk, dtype=np.float32),
                "wv": np.ascontiguousarray(wv, dtype=np.float32),
                "wp": wp,
                "identb": identb,
                "maskb": maskb,
            }
        )
    return in_maps


def kernel(x, W_attn, b_attn, W_proj, b_proj, _want_results=False, _spmd_kwargs=None):
    x = np.asarray(x, dtype=np.float32)
    W_attn = np.asarray(W_attn, dtype=np.float32)
    b_attn = np.asarray(b_attn, dtype=np.float32)
    W_proj = np.asarray(W_proj, dtype=np.float32)
    b_proj = np.asarray(b_proj, dtype=np.float32)

    if "nc" not in _CACHE:
        _CACHE["nc"] = _build()
    nc = _CACHE["nc"]

    in_maps = _prep_inputs(x, W_attn, b_attn, W_proj)
    kw = dict(_spmd_kwargs or {})
    res = run_bass_kernel_spmd(nc, in_maps, list(range(NCORES)), **kw)

    out = np.zeros((B, T, C), dtype=np.float32)
    for core in range(NCORES):
        out[core // 4] += res.results[core]["out"]
    out += b_proj[None, None, :]
    if _want_results:
        return out, res
    return out
